# revision 3
# baseline (speedup 1.0000x reference)
"""Trainium2 Bass kernel for GroupNorm + 1x1-conv guided cross-attention.

Computes, per batch element b:
    normed = GroupNorm32(input[b])                      # [128, 4096]
    k, v   = (wkv @ normed).split(2)                    # [64, 4096] each
    q      = wq @ quary[b]                              # [64, 4096]
    attn   = softmax((q.T @ k) / 8, axis=kv)            # [4096, 4096]
    out    = wout @ (v @ attn.T) + bout + input[b]      # [128, 4096]

Sharding: 8 cores = (batch b, query-row half). Each core holds the full
context feature map input[b] (needed for GroupNorm stats and k/v) and half
the query positions. kv positions are order-invariant (fully contracted
through softmax), so each core receives input[b] with its own spatial half
first - the residual is then always columns [0:2048] of the permuted input,
keeping the SPMD program uniform across cores.

On-core layout: channels on partitions, spatial positions on the free dim.
Attention scores are computed transposed (kv positions on partitions) so the
softmax denominator and the attn @ v contraction are both plain matmuls; the
softmax skips max-subtraction (scores are ~N(0,1); fp32 exp is exact to
2 ulp and cannot overflow until |s| > 88) and folds the 1/sumexp
normalization in after the attention-output matmul via a ones-row appended
to v^T (giving sumexp for free) and a K=1 broadcast matmul.
"""

import math
from contextlib import ExitStack

import numpy as np

import concourse.bass as bass
import concourse.mybir as mybir
import concourse.tile as tile
from concourse import bacc
from concourse.bass import ts
from concourse.bass_utils import run_bass_kernel_spmd

f32 = mybir.dt.float32
AF = mybir.ActivationFunctionType
ALU = mybir.AluOpType

B, CIN, CQ, H, W, GRP = 4, 128, 64, 64, 64, 32
HW = H * W          # 4096 spatial positions
SQ = HW // 2        # 2048 query positions per core
NK = HW // 128      # 32 kv tiles of 128 positions
N_CORES = 8
EPS = 1e-5
SCALE = 1.0 / math.sqrt(CQ)

_CACHE = {}
LAST_EXEC_NS = None


def _build():
    nc = bacc.Bacc("TRN2", target_bir_lowering=False, debug=False,
                   enable_asserts=True)

    x_d = nc.dram_tensor("x", [CIN, HW], f32, kind="ExternalInput")
    qry_d = nc.dram_tensor("qry", [CQ, SQ], f32, kind="ExternalInput")
    wkvT_d = nc.dram_tensor("wkvT", [CIN, 2 * CQ], f32, kind="ExternalInput")
    wqT_d = nc.dram_tensor("wqT", [CQ, CQ], f32, kind="ExternalInput")
    woutT_d = nc.dram_tensor("woutT", [CQ, CIN], f32, kind="ExternalInput")
    gam_d = nc.dram_tensor("gam", [CIN, 1], f32, kind="ExternalInput")
    bet_d = nc.dram_tensor("bet", [CIN, 1], f32, kind="ExternalInput")
    bout_d = nc.dram_tensor("bout", [CIN, 1], f32, kind="ExternalInput")
    gmap_d = nc.dram_tensor("gmap", [CIN, GRP], f32, kind="ExternalInput")
    gmapT_d = nc.dram_tensor("gmapT", [GRP, CIN], f32, kind="ExternalInput")
    ones_d = nc.dram_tensor("ones", [1, CQ], f32, kind="ExternalInput")
    out_d = nc.dram_tensor("out", [CIN, SQ], f32, kind="ExternalOutput")

    with tile.TileContext(nc) as tc:
        with ExitStack() as ctx:
            sp = ctx.enter_context(tc.tile_pool(name="singles", bufs=1))
            e_pool = ctx.enter_context(tc.tile_pool(name="e", bufs=2))
            epi_sp = ctx.enter_context(tc.tile_pool(name="episb", bufs=2))
            out_pool = ctx.enter_context(tc.tile_pool(name="osb", bufs=2))

            x_sb = sp.tile([CIN, HW], f32)
            nc.sync.dma_start(x_sb[:], x_d.ap())
            qry_sb = sp.tile([CQ, SQ], f32)
            nc.sync.dma_start(qry_sb[:], qry_d.ap())
            wkvT_sb = sp.tile([CIN, 2 * CQ], f32)
            nc.sync.dma_start(wkvT_sb[:], wkvT_d.ap())
            wqT_sb = sp.tile([CQ, CQ], f32)
            nc.sync.dma_start(wqT_sb[:], wqT_d.ap())
            woutT_sb = sp.tile([CQ, CIN], f32)
            nc.sync.dma_start(woutT_sb[:], woutT_d.ap())
            gam_sb = sp.tile([CIN, 1], f32)
            nc.sync.dma_start(gam_sb[:], gam_d.ap())
            bet_sb = sp.tile([CIN, 1], f32)
            nc.sync.dma_start(bet_sb[:], bet_d.ap())
            bout_sb = sp.tile([CIN, 1], f32)
            nc.sync.dma_start(bout_sb[:], bout_d.ap())
            gmap_sb = sp.tile([CIN, GRP], f32)
            nc.sync.dma_start(gmap_sb[:], gmap_d.ap())
            gmapT_sb = sp.tile([GRP, CIN], f32)
            nc.sync.dma_start(gmapT_sb[:], gmapT_d.ap())
            ones_sb = sp.tile([1, CQ], f32)
            nc.sync.dma_start(ones_sb[:], ones_d.ap())

            # ---- GroupNorm statistics ----
            # Per-channel mean/var via bn_stats (512-wide HW limit per call),
            # then combine channels into groups of 4 through one-hot matmuls:
            # group stats are means of (mu_c, E[x^2]_c) over the group.
            bn6 = sp.tile([CIN, 8, 6], f32)
            for i in range(8):
                nc.vector.bn_stats(bn6[:, i], x_sb[:, ts(i, 512)])
            cstat = sp.tile([CIN, 2], f32)
            nc.vector.bn_aggr(cstat[:], bn6[:])

            cm = sp.tile([CIN, 2], f32)       # (mu_c, E[x^2]_c)
            tmp1 = sp.tile([CIN, 1], f32)
            zero_sb = sp.tile([CIN, 1], f32)  # activation bias operand
            nc.vector.memset(zero_sb[:], 0.0)
            eps_sb = sp.tile([CIN, 1], f32)
            nc.vector.memset(eps_sb[:], EPS)
            nc.vector.tensor_copy(cm[:, 0:1], cstat[:, 0:1])
            nc.vector.tensor_mul(tmp1[:], cstat[:, 0:1], cstat[:, 0:1])
            nc.vector.tensor_add(cm[:, 1:2], cstat[:, 1:2], tmp1[:])

            with tc.tile_pool(name="pspre", bufs=3,
                              space=bass.MemorySpace.PSUM) as ps_pre:
                gps = ps_pre.tile([GRP, 2], f32, tag="pre")
                nc.tensor.matmul(gps[:], gmap_sb[:], cm[:], start=True, stop=True)
                gstat = sp.tile([GRP, 2], f32)
                nc.vector.tensor_scalar_mul(gstat[:], gps[:], 1.0 / (CIN // GRP))
                bps = ps_pre.tile([CIN, 2], f32, tag="pre")
                nc.tensor.matmul(bps[:], gmapT_sb[:], gstat[:], start=True, stop=True)
                bstat = sp.tile([CIN, 2], f32)  # (mu_g, E[x^2]_g) per channel
                nc.vector.tensor_copy(bstat[:], bps[:])

                # rstd = exp(-0.5*ln(var+eps)); Ln+Exp share one ACT table set
                var_t = sp.tile([CIN, 1], f32)
                nc.vector.tensor_mul(tmp1[:], bstat[:, 0:1], bstat[:, 0:1])
                nc.vector.tensor_sub(var_t[:], bstat[:, 1:2], tmp1[:])
                lnv = sp.tile([CIN, 1], f32)
                nc.scalar.activation(lnv[:], var_t[:], AF.Ln, bias=eps_sb[:])
                rstd = sp.tile([CIN, 1], f32)
                nc.scalar.activation(rstd[:], lnv[:], AF.Exp, bias=zero_sb[:], scale=-0.5)
                a_sb = sp.tile([CIN, 1], f32)   # normed = a*x + b per channel
                nc.vector.tensor_mul(a_sb[:], gam_sb[:], rstd[:])
                b_sb = sp.tile([CIN, 1], f32)
                nc.vector.tensor_mul(tmp1[:], bstat[:, 0:1], a_sb[:])
                nc.vector.tensor_sub(b_sb[:], bet_sb[:], tmp1[:])

                nrm = sp.tile([CIN, HW], f32)
                nc.vector.tensor_scalar(nrm[:], x_sb[:], a_sb[:], b_sb[:],
                                        ALU.mult, ALU.add)

                # residual (+ output bias), columns 0:SQ = this core's half
                res_sb = sp.tile([CIN, SQ], f32)
                nc.vector.tensor_scalar_add(res_sb[:], x_sb[:, 0:SQ], bout_sb[:])

                # ---- k, q, and v^T (with appended ones row) ----
                k_sb = sp.tile([CQ, HW], f32)
                for j in range(8):
                    kp = ps_pre.tile([CQ, 512], f32, tag="pre")
                    nc.tensor.matmul(kp[:], wkvT_sb[:, 0:CQ], nrm[:, ts(j, 512)],
                                     start=True, stop=True)
                    nc.vector.tensor_copy(k_sb[:, ts(j, 512)], kp[:])
                q_sb = sp.tile([CQ, SQ], f32)
                for j in range(4):
                    qp = ps_pre.tile([CQ, 512], f32, tag="pre")
                    nc.tensor.matmul(qp[:], wqT_sb[:], qry_sb[:, ts(j, 512)],
                                     start=True, stop=True)
                    nc.vector.tensor_copy(q_sb[:, ts(j, 512)], qp[:])
                # v^T[s, c] = sum_cin nrm[cin, s] * wvT[cin, c], one 128-row
                # chunk of s per matmul; column CQ holds 1.0 so the attention
                # matmul also produces the softmax denominator.
                vT_sb = sp.tile([128, NK, CQ + 1], f32)
                nc.vector.memset(vT_sb[:, :, CQ:CQ + 1], 1.0)
                for g in range(4):
                    vp = ps_pre.tile([128, 8, CQ], f32, tag="pre")
                    for i in range(8):
                        t = 8 * g + i
                        nc.tensor.matmul(vp[:, i], nrm[:, ts(t, 128)],
                                         wkvT_sb[:, CQ:2 * CQ],
                                         start=True, stop=True)
                    nc.vector.tensor_copy(vT_sb[:, 8 * g:8 * g + 8, 0:CQ], vp[:])

            # ---- attention main loop ----
            # S_T[sk, sq] tiles in PSUM -> exp on ScalarE -> accumulate
            # O_aug[c(+sum), sq] over all kv tiles. Two 1024-wide score
            # buffers per iteration keep ScalarE (the bottleneck) fed while
            # TensorE writes the next tile.
            with tc.tile_pool(name="psoacc", bufs=1,
                              space=bass.MemorySpace.PSUM) as ps_oacc:
                oacc = ps_oacc.tile([CQ + 1, SQ], f32)
                with tc.tile_pool(name="psst", bufs=1,
                                  space=bass.MemorySpace.PSUM) as ps_st:
                    for t in range(NK):
                        kt = k_sb[:, ts(t, 128)]
                        stA = ps_st.tile([128, 1024], f32, tag="stA")
                        stB = ps_st.tile([128, 1024], f32, tag="stB")
                        nc.tensor.matmul(stA[:, 0:512], kt, q_sb[:, 0:512],
                                         start=True, stop=True)
                        nc.tensor.matmul(stA[:, 512:1024], kt, q_sb[:, 512:1024],
                                         start=True, stop=True)
                        nc.tensor.matmul(stB[:, 0:512], kt, q_sb[:, 1024:1536],
                                         start=True, stop=True)
                        nc.tensor.matmul(stB[:, 512:1024], kt, q_sb[:, 1536:2048],
                                         start=True, stop=True)
                        et = e_pool.tile([128, SQ], f32)
                        nc.scalar.activation(et[:, 0:1024], stA[:], AF.Exp,
                                             bias=zero_sb[:], scale=SCALE)
                        nc.scalar.activation(et[:, 1024:2048], stB[:], AF.Exp,
                                             bias=zero_sb[:], scale=SCALE)
                        vt = vT_sb[:, t, :]
                        for j in range(4):
                            nc.tensor.matmul(oacc[:, ts(j, 512)], vt,
                                             et[:, ts(j, 512)],
                                             start=(t == 0), stop=(t == NK - 1))

                # ---- epilogue: normalize, project, add residual ----
                with tc.tile_pool(name="psepi", bufs=2,
                                  space=bass.MemorySpace.PSUM) as ps_epi:
                    for j in range(4):
                        r_sb = epi_sp.tile([1, 512], f32, tag="r")
                        nc.vector.reciprocal(r_sb[:], oacc[CQ:CQ + 1, ts(j, 512)])
                        rb = ps_epi.tile([CQ, 512], f32, tag="rb")
                        nc.tensor.matmul(rb[:], ones_sb[:], r_sb[:],
                                         start=True, stop=True)
                        o_sb = epi_sp.tile([CQ, 512], f32, tag="o")
                        nc.scalar.copy(o_sb[:], oacc[0:CQ, ts(j, 512)])
                        on_sb = epi_sp.tile([CQ, 512], f32, tag="on")
                        nc.vector.tensor_mul(on_sb[:], o_sb[:], rb[:])
                        fp = ps_epi.tile([CIN, 512], f32, tag="f")
                        nc.tensor.matmul(fp[:], woutT_sb[:], on_sb[:],
                                         start=True, stop=True)
                        ot = out_pool.tile([CIN, 512], f32)
                        nc.vector.tensor_add(ot[:], fp[:], res_sb[:, ts(j, 512)])
                        nc.sync.dma_start(out_d.ap()[:, ts(j, 512)], ot[:])

    nc.compile()
    return nc


def get_nc():
    if "nc" not in _CACHE:
        _CACHE["nc"] = _build()
    return _CACHE["nc"]


def make_in_maps(input, quary, gn_gamma, gn_beta, wq, wkv, wout, bout):
    input = np.ascontiguousarray(np.asarray(input, dtype=np.float32))
    quary = np.ascontiguousarray(np.asarray(quary, dtype=np.float32))
    wkvT = np.ascontiguousarray(np.asarray(wkv, dtype=np.float32).T)
    wqT = np.ascontiguousarray(np.asarray(wq, dtype=np.float32).T)
    woutT = np.ascontiguousarray(np.asarray(wout, dtype=np.float32).T)
    gam = np.ascontiguousarray(np.asarray(gn_gamma, np.float32).reshape(CIN, 1))
    bet = np.ascontiguousarray(np.asarray(gn_beta, np.float32).reshape(CIN, 1))
    bo = np.ascontiguousarray(np.asarray(bout, np.float32).reshape(CIN, 1))
    gmap = np.zeros((CIN, GRP), np.float32)
    gmap[np.arange(CIN), np.arange(CIN) // (CIN // GRP)] = 1.0
    gmapT = np.ascontiguousarray(gmap.T)
    ones = np.ones((1, CQ), np.float32)

    in_maps = []
    for core in range(N_CORES):
        b, half = divmod(core, 2)
        xb = input[b].reshape(CIN, 2, SQ)
        x_perm = np.ascontiguousarray(
            np.concatenate([xb[:, half], xb[:, 1 - half]], axis=1))
        qh = np.ascontiguousarray(quary[b].reshape(CQ, 2, SQ)[:, half])
        in_maps.append(dict(x=x_perm, qry=qh, wkvT=wkvT, wqT=wqT, woutT=woutT,
                            gam=gam, bet=bet, bout=bo, gmap=gmap, gmapT=gmapT,
                            ones=ones))
    return in_maps


def gather_out(results):
    out = np.empty((B, CIN, HW), np.float32)
    for core in range(N_CORES):
        b, half = divmod(core, 2)
        out[b, :, half * SQ:(half + 1) * SQ] = results[core]["out"]
    return out.reshape(B, CIN, H, W)


def kernel(input, quary, gn_gamma, gn_beta, wq, wkv, wout, bout,
           _trace=False):
    global LAST_EXEC_NS
    nc = get_nc()
    in_maps = make_in_maps(input, quary, gn_gamma, gn_beta, wq, wkv, wout, bout)
    res = run_bass_kernel_spmd(nc, in_maps, list(range(N_CORES)), trace=_trace)
    LAST_EXEC_NS = res.exec_time_ns
    return gather_out(res.results)


# revision 6
# speedup vs baseline: 2.3194x; 2.3194x over previous
"""Trainium2 Bass kernel for GroupNorm + 1x1-conv guided cross-attention.

Computes, per batch element b:
    normed = GroupNorm32(input[b])                      # [128, 4096]
    k, v   = (wkv @ normed).split(2)                    # [64, 4096] each
    q      = wq @ quary[b]                              # [64, 4096]
    attn   = softmax((q.T @ k) / 8, axis=kv)            # [4096, 4096]
    out    = wout @ (v @ attn.T) + bout + input[b]      # [128, 4096]

Sharding: 8 cores = (batch b, query-row half). Each core holds the full
context feature map input[b] (needed for GroupNorm stats and k/v) and half
the query positions. kv positions are order-invariant (fully contracted
through softmax), so each core receives input[b] with its own spatial half
first - the residual is then always columns [0:2048] of the permuted input,
keeping the SPMD program uniform across cores.

On-core layout: channels on partitions, spatial positions on the free dim.
Attention scores are computed transposed (kv positions on partitions) so the
softmax denominator and the attn @ v contraction are both plain matmuls; the
softmax skips max-subtraction (scores are ~N(0,1); fp32 exp is exact to
2 ulp and cannot overflow until |s| > 88) and folds the 1/sumexp
normalization in after the attention-output matmul via a ones-row appended
to v^T (giving sumexp for free) and a K=1 broadcast matmul.
"""

import math
from contextlib import ExitStack

import numpy as np

import concourse.bass as bass
import concourse.mybir as mybir
import concourse.tile as tile
from concourse import bacc
from concourse.bass import ts
from concourse.bass_utils import run_bass_kernel_spmd

f32 = mybir.dt.float32
f32r = mybir.dt.float32r
AF = mybir.ActivationFunctionType
ALU = mybir.AluOpType

B, CIN, CQ, H, W, GRP = 4, 128, 64, 64, 64, 32
HW = H * W          # 4096 spatial positions
SQ = HW // 2        # 2048 query positions per core
NK = HW // 128      # 32 kv tiles of 128 positions
N_CORES = 8
EPS = 1e-5
SCALE = 1.0 / math.sqrt(CQ)

_CACHE = {}
LAST_EXEC_NS = None


def _build():
    nc = bacc.Bacc("TRN2", target_bir_lowering=False, debug=False,
                   enable_asserts=True)

    x_d = nc.dram_tensor("x", [CIN, HW], f32, kind="ExternalInput")
    qry_d = nc.dram_tensor("qry", [CQ, SQ], f32, kind="ExternalInput")
    wkvT_d = nc.dram_tensor("wkvT", [CIN, 2 * CQ], f32, kind="ExternalInput")
    wqT_d = nc.dram_tensor("wqT", [CQ, CQ], f32, kind="ExternalInput")
    woutT_d = nc.dram_tensor("woutT", [CQ, CIN], f32, kind="ExternalInput")
    gam_d = nc.dram_tensor("gam", [CIN, 1], f32, kind="ExternalInput")
    bet_d = nc.dram_tensor("bet", [CIN, 1], f32, kind="ExternalInput")
    bout_d = nc.dram_tensor("bout", [CIN, 1], f32, kind="ExternalInput")
    gmap_d = nc.dram_tensor("gmap", [CIN, GRP], f32, kind="ExternalInput")
    gmapT_d = nc.dram_tensor("gmapT", [GRP, CIN], f32, kind="ExternalInput")
    out_d = nc.dram_tensor("out", [CIN, SQ], f32, kind="ExternalOutput")

    with tile.TileContext(nc) as tc:
        with ExitStack() as ctx, nc.allow_low_precision(
                reason="f32r is fp32-range with ~19-bit mantissa; matmul "
                       "accumulation stays fp32 in PSUM"):
            sp = ctx.enter_context(tc.tile_pool(name="singles", bufs=1))
            e_pool = ctx.enter_context(tc.tile_pool(name="e", bufs=2))
            epi_sp = ctx.enter_context(tc.tile_pool(name="episb", bufs=2))
            out_pool = ctx.enter_context(tc.tile_pool(name="osb", bufs=2))

            x_sb = sp.tile([CIN, HW], f32)
            nc.sync.dma_start(x_sb[:], x_d.ap())
            qry_sb = sp.tile([CQ, SQ], f32)
            nc.sync.dma_start(qry_sb[:], qry_d.ap())
            wkvT_sb = sp.tile([CIN, 2 * CQ], f32)
            nc.sync.dma_start(wkvT_sb[:], wkvT_d.ap())
            wqT_sb = sp.tile([CQ, CQ], f32)
            nc.sync.dma_start(wqT_sb[:], wqT_d.ap())
            woutT_sb = sp.tile([CQ, CIN], f32)
            nc.sync.dma_start(woutT_sb[:], woutT_d.ap())
            gam_sb = sp.tile([CIN, 1], f32)
            nc.sync.dma_start(gam_sb[:], gam_d.ap())
            bet_sb = sp.tile([CIN, 1], f32)
            nc.sync.dma_start(bet_sb[:], bet_d.ap())
            bout_sb = sp.tile([CIN, 1], f32)
            nc.sync.dma_start(bout_sb[:], bout_d.ap())
            gmap_sb = sp.tile([CIN, GRP], f32)
            nc.sync.dma_start(gmap_sb[:], gmap_d.ap())
            gmapT_sb = sp.tile([GRP, CIN], f32)
            nc.sync.dma_start(gmapT_sb[:], gmapT_d.ap())
            ones_sb = sp.tile([1, CQ], f32r)
            nc.vector.memset(ones_sb[:].bitcast(mybir.dt.uint32), 0x3F800000)
            # f32r-rounded weight copies for full-rate PE matmuls
            wkvT_r = sp.tile([CIN, 2 * CQ], f32r)
            nc.vector.tensor_copy(wkvT_r[:], wkvT_sb[:])
            wqT_r = sp.tile([CQ, CQ], f32r)
            nc.vector.tensor_copy(wqT_r[:], wqT_sb[:])
            woutT_r = sp.tile([CQ, CIN], f32r)
            nc.vector.tensor_copy(woutT_r[:], woutT_sb[:])
            qry_r = sp.tile([CQ, SQ], f32r)
            nc.vector.tensor_copy(qry_r[:], qry_sb[:])

            # ---- GroupNorm statistics ----
            # Per-channel mean/var via bn_stats (512-wide HW limit per call),
            # then combine channels into groups of 4 through one-hot matmuls:
            # group stats are means of (mu_c, E[x^2]_c) over the group.
            bn6 = sp.tile([CIN, 8, 6], f32)
            for i in range(8):
                nc.vector.bn_stats(bn6[:, i], x_sb[:, ts(i, 512)])
            cstat = sp.tile([CIN, 2], f32)
            nc.vector.bn_aggr(cstat[:], bn6[:])

            cm = sp.tile([CIN, 2], f32)       # (mu_c, E[x^2]_c)
            tmp1 = sp.tile([CIN, 1], f32)
            zero_sb = sp.tile([CIN, 1], f32)  # activation bias operand
            nc.vector.memset(zero_sb[:], 0.0)
            eps_sb = sp.tile([CIN, 1], f32)
            nc.vector.memset(eps_sb[:], EPS)
            nc.vector.tensor_copy(cm[:, 0:1], cstat[:, 0:1])
            nc.vector.tensor_mul(tmp1[:], cstat[:, 0:1], cstat[:, 0:1])
            nc.vector.tensor_add(cm[:, 1:2], cstat[:, 1:2], tmp1[:])

            with tc.tile_pool(name="pspre", bufs=3,
                              space=bass.MemorySpace.PSUM) as ps_pre:
                gps = ps_pre.tile([GRP, 2], f32, tag="pre")
                nc.tensor.matmul(gps[:], gmap_sb[:], cm[:], start=True, stop=True)
                gstat = sp.tile([GRP, 2], f32)
                nc.vector.tensor_scalar_mul(gstat[:], gps[:], 1.0 / (CIN // GRP))
                bps = ps_pre.tile([CIN, 2], f32, tag="pre")
                nc.tensor.matmul(bps[:], gmapT_sb[:], gstat[:], start=True, stop=True)
                bstat = sp.tile([CIN, 2], f32)  # (mu_g, E[x^2]_g) per channel
                nc.vector.tensor_copy(bstat[:], bps[:])

                # rstd = exp(-0.5*ln(var+eps)); Ln+Exp share one ACT table set
                var_t = sp.tile([CIN, 1], f32)
                nc.vector.tensor_mul(tmp1[:], bstat[:, 0:1], bstat[:, 0:1])
                nc.vector.tensor_sub(var_t[:], bstat[:, 1:2], tmp1[:])
                lnv = sp.tile([CIN, 1], f32)
                nc.scalar.activation(lnv[:], var_t[:], AF.Ln, bias=eps_sb[:])
                rstd = sp.tile([CIN, 1], f32)
                nc.scalar.activation(rstd[:], lnv[:], AF.Exp, bias=zero_sb[:], scale=-0.5)
                a_sb = sp.tile([CIN, 1], f32)   # normed = a*x + b per channel
                nc.vector.tensor_mul(a_sb[:], gam_sb[:], rstd[:])
                b_sb = sp.tile([CIN, 1], f32)
                nc.vector.tensor_mul(tmp1[:], bstat[:, 0:1], a_sb[:])
                nc.vector.tensor_sub(b_sb[:], bet_sb[:], tmp1[:])

                nrm = sp.tile([CIN, HW], f32r)
                nc.vector.tensor_scalar(nrm[:], x_sb[:], a_sb[:], b_sb[:],
                                        ALU.mult, ALU.add)

                # residual (+ output bias), columns 0:SQ = this core's half
                res_sb = sp.tile([CIN, SQ], f32)
                nc.vector.tensor_scalar_add(res_sb[:], x_sb[:, 0:SQ], bout_sb[:])

                # ---- k, q, and v^T (with appended ones row) ----
                k_sb = sp.tile([CQ, HW], f32r)
                for j in range(8):
                    kp = ps_pre.tile([CQ, 512], f32, tag="pre")
                    nc.tensor.matmul(kp[:], wkvT_r[:, 0:CQ], nrm[:, ts(j, 512)],
                                     start=True, stop=True)
                    nc.vector.tensor_copy(k_sb[:, ts(j, 512)], kp[:])
                q_sb = sp.tile([CQ, SQ], f32r)
                for j in range(4):
                    qp = ps_pre.tile([CQ, 512], f32, tag="pre")
                    nc.tensor.matmul(qp[:], wqT_r[:], qry_r[:, ts(j, 512)],
                                     start=True, stop=True)
                    nc.vector.tensor_copy(q_sb[:, ts(j, 512)], qp[:])
                # v^T[s, c] = sum_cin nrm[cin, s] * wvT[cin, c], one 128-row
                # chunk of s per matmul; column CQ holds 1.0 so the attention
                # matmul also produces the softmax denominator.
                vT_sb = sp.tile([128, NK, CQ + 1], f32r)
                nc.vector.memset(vT_sb[:, :, CQ:CQ + 1].bitcast(mybir.dt.uint32), 0x3F800000)
                for g in range(4):
                    vp = ps_pre.tile([128, 8, CQ], f32, tag="pre")
                    for i in range(8):
                        t = 8 * g + i
                        nc.tensor.matmul(vp[:, i], nrm[:, ts(t, 128)],
                                         wkvT_r[:, CQ:2 * CQ],
                                         start=True, stop=True)
                    nc.vector.tensor_copy(vT_sb[:, 8 * g:8 * g + 8, 0:CQ], vp[:])

            # ---- attention main loop ----
            # S_T[sk, sq] tiles in PSUM -> exp on ScalarE -> accumulate
            # O_aug[c(+sum), sq] over all kv tiles. Two 1024-wide score
            # buffers per iteration keep ScalarE (the bottleneck) fed while
            # TensorE writes the next tile.
            with tc.tile_pool(name="psoacc", bufs=1,
                              space=bass.MemorySpace.PSUM) as ps_oacc:
                oacc = ps_oacc.tile([CQ + 1, SQ], f32)
                with tc.tile_pool(name="psst", bufs=1,
                                  space=bass.MemorySpace.PSUM) as ps_st:
                    for t in range(NK):
                        kt = k_sb[:, ts(t, 128)]
                        stA = ps_st.tile([128, 1024], f32, tag="stA")
                        stB = ps_st.tile([128, 1024], f32, tag="stB")
                        nc.tensor.matmul(stA[:, 0:512], kt, q_sb[:, 0:512],
                                         start=True, stop=True)
                        nc.tensor.matmul(stA[:, 512:1024], kt, q_sb[:, 512:1024],
                                         start=True, stop=True)
                        nc.tensor.matmul(stB[:, 0:512], kt, q_sb[:, 1024:1536],
                                         start=True, stop=True)
                        nc.tensor.matmul(stB[:, 512:1024], kt, q_sb[:, 1536:2048],
                                         start=True, stop=True)
                        et = e_pool.tile([128, SQ], f32r)
                        nc.scalar.activation(et[:, 0:1024], stA[:], AF.Exp,
                                             bias=zero_sb[:], scale=SCALE)
                        nc.scalar.activation(et[:, 1024:2048], stB[:], AF.Exp,
                                             bias=zero_sb[:], scale=SCALE)
                        vt = vT_sb[:, t, :]
                        for j in range(4):
                            nc.tensor.matmul(oacc[:, ts(j, 512)], vt,
                                             et[:, ts(j, 512)],
                                             start=(t == 0), stop=(t == NK - 1))

                # ---- epilogue: normalize, project, add residual ----
                with tc.tile_pool(name="psepi", bufs=2,
                                  space=bass.MemorySpace.PSUM) as ps_epi:
                    for j in range(4):
                        r_sb = epi_sp.tile([1, 512], f32r, tag="r")
                        nc.vector.reciprocal(r_sb[:], oacc[CQ:CQ + 1, ts(j, 512)])
                        rb = ps_epi.tile([CQ, 512], f32, tag="rb")
                        nc.tensor.matmul(rb[:], ones_sb[:], r_sb[:],
                                         start=True, stop=True)
                        o_sb = epi_sp.tile([CQ, 512], f32, tag="o")
                        nc.scalar.copy(o_sb[:], oacc[0:CQ, ts(j, 512)])
                        on_sb = epi_sp.tile([CQ, 512], f32r, tag="on")
                        nc.vector.tensor_mul(on_sb[:], o_sb[:], rb[:])
                        fp = ps_epi.tile([CIN, 512], f32, tag="f")
                        nc.tensor.matmul(fp[:], woutT_r[:], on_sb[:],
                                         start=True, stop=True)
                        ot = out_pool.tile([CIN, 512], f32)
                        nc.vector.tensor_add(ot[:], fp[:], res_sb[:, ts(j, 512)])
                        nc.sync.dma_start(out_d.ap()[:, ts(j, 512)], ot[:])

    nc.compile()
    return nc


def get_nc():
    if "nc" not in _CACHE:
        _CACHE["nc"] = _build()
    return _CACHE["nc"]


def make_in_maps(input, quary, gn_gamma, gn_beta, wq, wkv, wout, bout):
    input = np.ascontiguousarray(np.asarray(input, dtype=np.float32))
    quary = np.ascontiguousarray(np.asarray(quary, dtype=np.float32))
    wkvT = np.ascontiguousarray(np.asarray(wkv, dtype=np.float32).T)
    wqT = np.ascontiguousarray(np.asarray(wq, dtype=np.float32).T)
    woutT = np.ascontiguousarray(np.asarray(wout, dtype=np.float32).T)
    gam = np.ascontiguousarray(np.asarray(gn_gamma, np.float32).reshape(CIN, 1))
    bet = np.ascontiguousarray(np.asarray(gn_beta, np.float32).reshape(CIN, 1))
    bo = np.ascontiguousarray(np.asarray(bout, np.float32).reshape(CIN, 1))
    gmap = np.zeros((CIN, GRP), np.float32)
    gmap[np.arange(CIN), np.arange(CIN) // (CIN // GRP)] = 1.0
    gmapT = np.ascontiguousarray(gmap.T)

    in_maps = []
    for core in range(N_CORES):
        b, half = divmod(core, 2)
        xb = input[b].reshape(CIN, 2, SQ)
        x_perm = np.ascontiguousarray(
            np.concatenate([xb[:, half], xb[:, 1 - half]], axis=1))
        qh = np.ascontiguousarray(quary[b].reshape(CQ, 2, SQ)[:, half])
        in_maps.append(dict(x=x_perm, qry=qh, wkvT=wkvT, wqT=wqT, woutT=woutT,
                            gam=gam, bet=bet, bout=bo, gmap=gmap,
                            gmapT=gmapT))
    return in_maps


def gather_out(results):
    out = np.empty((B, CIN, HW), np.float32)
    for core in range(N_CORES):
        b, half = divmod(core, 2)
        out[b, :, half * SQ:(half + 1) * SQ] = results[core]["out"]
    return out.reshape(B, CIN, H, W)


def kernel(input, quary, gn_gamma, gn_beta, wq, wkv, wout, bout,
           _trace=False):
    global LAST_EXEC_NS
    nc = get_nc()
    in_maps = make_in_maps(input, quary, gn_gamma, gn_beta, wq, wkv, wout, bout)
    res = run_bass_kernel_spmd(nc, in_maps, list(range(N_CORES)), trace=_trace)
    LAST_EXEC_NS = res.exec_time_ns
    return gather_out(res.results)


# revision 7
# speedup vs baseline: 2.4020x; 1.0356x over previous
"""Trainium2 Bass kernel for GroupNorm + 1x1-conv guided cross-attention.

Computes, per batch element b:
    normed = GroupNorm32(input[b])                      # [128, 4096]
    k, v   = (wkv @ normed).split(2)                    # [64, 4096] each
    q      = wq @ quary[b]                              # [64, 4096]
    attn   = softmax((q.T @ k) / 8, axis=kv)            # [4096, 4096]
    out    = wout @ (v @ attn.T) + bout + input[b]      # [128, 4096]

Sharding: 8 cores = (batch b, query-row half). Each core holds the full
context feature map input[b] (needed for GroupNorm stats and k/v) and half
the query positions. kv positions are order-invariant (fully contracted
through softmax), so each core receives input[b] with its own spatial half
first - the residual is then always columns [0:2048] of the permuted input,
keeping the SPMD program uniform across cores.

On-core layout: channels on partitions, spatial positions on the free dim.
Attention scores are computed transposed (kv positions on partitions) so the
softmax denominator and the attn @ v contraction are both plain matmuls; the
softmax skips max-subtraction (scores are ~N(0,1); fp32 exp is exact to
2 ulp and cannot overflow until |s| > 88) and folds the 1/sumexp
normalization in after the attention-output matmul via a ones-row appended
to v^T (giving sumexp for free) and a K=1 broadcast matmul.
"""

import math
from contextlib import ExitStack

import numpy as np

import concourse.bass as bass
import concourse.mybir as mybir
import concourse.tile as tile
from concourse import bacc
from concourse.bass import ts
from concourse.bass_utils import run_bass_kernel_spmd

f32 = mybir.dt.float32
f32r = mybir.dt.float32r
AF = mybir.ActivationFunctionType
ALU = mybir.AluOpType

B, CIN, CQ, H, W, GRP = 4, 128, 64, 64, 64, 32
HW = H * W          # 4096 spatial positions
SQ = HW // 2        # 2048 query positions per core
NK = HW // 128      # 32 kv tiles of 128 positions
N_CORES = 8
EPS = 1e-5
SCALE = 1.0 / math.sqrt(CQ)

_CACHE = {}
LAST_EXEC_NS = None


def _build():
    nc = bacc.Bacc("TRN2", target_bir_lowering=False, debug=False,
                   enable_asserts=True)

    x_d = nc.dram_tensor("x", [CIN, HW], f32, kind="ExternalInput")
    qry_d = nc.dram_tensor("qry", [CQ, SQ], f32, kind="ExternalInput")
    wkvT_d = nc.dram_tensor("wkvT", [CIN, 2 * CQ], f32, kind="ExternalInput")
    wqT_d = nc.dram_tensor("wqT", [CQ, CQ], f32, kind="ExternalInput")
    woutT_d = nc.dram_tensor("woutT", [CQ, CIN], f32, kind="ExternalInput")
    gam_d = nc.dram_tensor("gam", [CIN, 1], f32, kind="ExternalInput")
    bet_d = nc.dram_tensor("bet", [CIN, 1], f32, kind="ExternalInput")
    bout_d = nc.dram_tensor("bout", [CIN, 1], f32, kind="ExternalInput")
    gmap_d = nc.dram_tensor("gmap", [CIN, GRP], f32, kind="ExternalInput")
    gmapT_d = nc.dram_tensor("gmapT", [GRP, CIN], f32, kind="ExternalInput")
    out_d = nc.dram_tensor("out", [CIN, SQ], f32, kind="ExternalOutput")

    with tile.TileContext(nc) as tc:
        with ExitStack() as ctx, nc.allow_low_precision(
                reason="f32r is fp32-range with ~19-bit mantissa; matmul "
                       "accumulation stays fp32 in PSUM"):
            sp = ctx.enter_context(tc.tile_pool(name="singles", bufs=1))
            e_pool = ctx.enter_context(tc.tile_pool(name="e", bufs=3))
            epi_sp = ctx.enter_context(tc.tile_pool(name="episb", bufs=2))
            out_pool = ctx.enter_context(tc.tile_pool(name="osb", bufs=2))

            x_sb = sp.tile([CIN, HW], f32)
            for i in range(4):
                nc.sync.dma_start(x_sb[:, ts(i, HW // 4)], x_d.ap()[:, ts(i, HW // 4)])
            qry_sb = sp.tile([CQ, SQ], f32)
            nc.sync.dma_start(qry_sb[:], qry_d.ap())
            wkvT_sb = sp.tile([CIN, 2 * CQ], f32)
            nc.sync.dma_start(wkvT_sb[:], wkvT_d.ap())
            wqT_sb = sp.tile([CQ, CQ], f32)
            nc.sync.dma_start(wqT_sb[:], wqT_d.ap())
            woutT_sb = sp.tile([CQ, CIN], f32)
            nc.sync.dma_start(woutT_sb[:], woutT_d.ap())
            gam_sb = sp.tile([CIN, 1], f32)
            nc.sync.dma_start(gam_sb[:], gam_d.ap())
            bet_sb = sp.tile([CIN, 1], f32)
            nc.sync.dma_start(bet_sb[:], bet_d.ap())
            bout_sb = sp.tile([CIN, 1], f32)
            nc.sync.dma_start(bout_sb[:], bout_d.ap())
            gmap_sb = sp.tile([CIN, GRP], f32)
            nc.sync.dma_start(gmap_sb[:], gmap_d.ap())
            gmapT_sb = sp.tile([GRP, CIN], f32)
            nc.sync.dma_start(gmapT_sb[:], gmapT_d.ap())
            ones_sb = sp.tile([1, CQ], f32r)
            nc.vector.memset(ones_sb[:].bitcast(mybir.dt.uint32), 0x3F800000)
            # f32r-rounded weight copies for full-rate PE matmuls
            wkvT_r = sp.tile([CIN, 2 * CQ], f32r)
            nc.vector.tensor_copy(wkvT_r[:], wkvT_sb[:])
            wqT_r = sp.tile([CQ, CQ], f32r)
            nc.vector.tensor_copy(wqT_r[:], wqT_sb[:])
            woutT_r = sp.tile([CQ, CIN], f32r)
            nc.vector.tensor_copy(woutT_r[:], woutT_sb[:])
            qry_r = sp.tile([CQ, SQ], f32r)
            nc.vector.tensor_copy(qry_r[:], qry_sb[:])

            # ---- GroupNorm statistics ----
            # Per-channel mean/var via bn_stats (512-wide HW limit per call),
            # then combine channels into groups of 4 through one-hot matmuls:
            # group stats are means of (mu_c, E[x^2]_c) over the group.
            bn6 = sp.tile([CIN, 8, 6], f32)
            for i in range(8):
                nc.vector.bn_stats(bn6[:, i], x_sb[:, ts(i, 512)])
            cstat = sp.tile([CIN, 2], f32)
            nc.vector.bn_aggr(cstat[:], bn6[:])

            cm = sp.tile([CIN, 2], f32)       # (mu_c, E[x^2]_c)
            tmp1 = sp.tile([CIN, 1], f32)
            zero_sb = sp.tile([CIN, 1], f32)  # activation bias operand
            nc.vector.memset(zero_sb[:], 0.0)
            eps_sb = sp.tile([CIN, 1], f32)
            nc.vector.memset(eps_sb[:], EPS)
            warm = sp.tile([1, 1], f32)
            nc.scalar.activation(warm[:], zero_sb[0:1, :], AF.Exp,
                                 bias=zero_sb[0:1, :])
            nc.vector.tensor_copy(cm[:, 0:1], cstat[:, 0:1])
            nc.vector.tensor_mul(tmp1[:], cstat[:, 0:1], cstat[:, 0:1])
            nc.vector.tensor_add(cm[:, 1:2], cstat[:, 1:2], tmp1[:])

            with tc.tile_pool(name="pspre", bufs=3,
                              space=bass.MemorySpace.PSUM) as ps_pre:
                gps = ps_pre.tile([GRP, 2], f32, tag="pre")
                nc.tensor.matmul(gps[:], gmap_sb[:], cm[:], start=True, stop=True)
                gstat = sp.tile([GRP, 2], f32)
                nc.vector.tensor_scalar_mul(gstat[:], gps[:], 1.0 / (CIN // GRP))
                bps = ps_pre.tile([CIN, 2], f32, tag="pre")
                nc.tensor.matmul(bps[:], gmapT_sb[:], gstat[:], start=True, stop=True)
                bstat = sp.tile([CIN, 2], f32)  # (mu_g, E[x^2]_g) per channel
                nc.vector.tensor_copy(bstat[:], bps[:])

                # var = E[x^2] - mu^2, then rstd = 1/sqrt(var+eps) via the
                # bit-trick seed + 3 Newton steps, all on VectorE (keeps the
                # Scalar engine free for exp and avoids a 2nd ACT table set)
                var_t = sp.tile([CIN, 1], f32)
                nc.vector.tensor_mul(tmp1[:], bstat[:, 0:1], bstat[:, 0:1])
                nc.vector.tensor_sub(var_t[:], bstat[:, 1:2], tmp1[:])
                nc.vector.tensor_scalar_add(var_t[:], var_t[:], EPS)
                magic = sp.tile([CIN, 1], mybir.dt.uint32)
                nc.vector.memset(magic[:], 0x5F3759DF)
                half_v = sp.tile([CIN, 1], f32)
                nc.vector.tensor_scalar_mul(half_v[:], var_t[:], 0.5)
                rstd = sp.tile([CIN, 1], f32)
                ri = sp.tile([CIN, 1], mybir.dt.uint32)
                nc.vector.tensor_scalar(ri[:], var_t[:].bitcast(mybir.dt.uint32),
                                        1, None, ALU.logical_shift_right)
                nc.vector.tensor_sub(rstd[:].bitcast(mybir.dt.uint32), magic[:], ri[:])
                t_a = sp.tile([CIN, 1], f32)
                for _ in range(3):
                    nc.vector.tensor_mul(t_a[:], rstd[:], rstd[:])
                    nc.vector.tensor_mul(t_a[:], t_a[:], half_v[:])
                    nc.vector.tensor_scalar(t_a[:], t_a[:], 1.5, -1.0,
                                            ALU.subtract, ALU.mult)
                    nc.vector.tensor_mul(rstd[:], rstd[:], t_a[:])
                a_sb = sp.tile([CIN, 1], f32)   # normed = a*x + b per channel
                nc.vector.tensor_mul(a_sb[:], gam_sb[:], rstd[:])
                b_sb = sp.tile([CIN, 1], f32)
                nc.vector.tensor_mul(tmp1[:], bstat[:, 0:1], a_sb[:])
                nc.vector.tensor_sub(b_sb[:], bet_sb[:], tmp1[:])

                nrm = sp.tile([CIN, HW], f32r)
                nc.vector.tensor_scalar(nrm[:], x_sb[:], a_sb[:], b_sb[:],
                                        ALU.mult, ALU.add)

                # residual (+ output bias), columns 0:SQ = this core's half
                res_sb = sp.tile([CIN, SQ], f32)
                nc.vector.tensor_scalar_add(res_sb[:], x_sb[:, 0:SQ], bout_sb[:])

                # ---- k, q, and v^T (with appended ones row) ----
                k_sb = sp.tile([CQ, HW], f32r)
                for j in range(8):
                    kp = ps_pre.tile([CQ, 512], f32, tag="pre")
                    nc.tensor.matmul(kp[:], wkvT_r[:, 0:CQ], nrm[:, ts(j, 512)],
                                     start=True, stop=True)
                    nc.scalar.copy(k_sb[:, ts(j, 512)], kp[:])
                q_sb = sp.tile([CQ, SQ], f32r)
                for j in range(4):
                    qp = ps_pre.tile([CQ, 512], f32, tag="pre")
                    nc.tensor.matmul(qp[:], wqT_r[:], qry_r[:, ts(j, 512)],
                                     start=True, stop=True)
                    nc.scalar.copy(q_sb[:, ts(j, 512)], qp[:])
                # v^T[s, c] = sum_cin nrm[cin, s] * wvT[cin, c], one 128-row
                # chunk of s per matmul; column CQ holds 1.0 so the attention
                # matmul also produces the softmax denominator.
                vT_sb = sp.tile([128, NK, CQ + 1], f32r)
                nc.vector.memset(vT_sb[:, :, CQ:CQ + 1].bitcast(mybir.dt.uint32), 0x3F800000)
                for g in range(4):
                    vp = ps_pre.tile([128, 8, CQ], f32, tag="pre")
                    for i in range(8):
                        t = 8 * g + i
                        nc.tensor.matmul(vp[:, i], nrm[:, ts(t, 128)],
                                         wkvT_r[:, CQ:2 * CQ],
                                         start=True, stop=True)
                    nc.vector.tensor_copy(vT_sb[:, 8 * g:8 * g + 8, 0:CQ], vp[:])

            # ---- attention main loop ----
            # S_T[sk, sq] tiles in PSUM -> exp on ScalarE -> accumulate
            # O_aug[c(+sum), sq] over all kv tiles. Two 1024-wide score
            # buffers per iteration keep ScalarE (the bottleneck) fed while
            # TensorE writes the next tile.
            with tc.tile_pool(name="psoacc", bufs=1,
                              space=bass.MemorySpace.PSUM) as ps_oacc:
                oacc = ps_oacc.tile([CQ + 1, SQ], f32)
                with tc.tile_pool(name="psst", bufs=1,
                                  space=bass.MemorySpace.PSUM) as ps_st:
                    for t in range(NK):
                        kt = k_sb[:, ts(t, 128)]
                        stA = ps_st.tile([128, 1024], f32, tag="stA")
                        stB = ps_st.tile([128, 1024], f32, tag="stB")
                        nc.tensor.matmul(stA[:, 0:512], kt, q_sb[:, 0:512],
                                         start=True, stop=True)
                        nc.tensor.matmul(stA[:, 512:1024], kt, q_sb[:, 512:1024],
                                         start=True, stop=True)
                        nc.tensor.matmul(stB[:, 0:512], kt, q_sb[:, 1024:1536],
                                         start=True, stop=True)
                        nc.tensor.matmul(stB[:, 512:1024], kt, q_sb[:, 1536:2048],
                                         start=True, stop=True)
                        et = e_pool.tile([128, SQ], f32r)
                        nc.scalar.activation(et[:, 0:1024], stA[:], AF.Exp,
                                             bias=zero_sb[:], scale=SCALE)
                        nc.scalar.activation(et[:, 1024:2048], stB[:], AF.Exp,
                                             bias=zero_sb[:], scale=SCALE)
                        vt = vT_sb[:, t, :]
                        for j in range(4):
                            nc.tensor.matmul(oacc[:, ts(j, 512)], vt,
                                             et[:, ts(j, 512)],
                                             start=(t == 0), stop=(t == NK - 1))

                # ---- epilogue: normalize, project, add residual ----
                with tc.tile_pool(name="psepi", bufs=2,
                                  space=bass.MemorySpace.PSUM) as ps_epi:
                    for j in range(4):
                        r_sb = epi_sp.tile([1, 512], f32r, tag="r")
                        nc.vector.reciprocal(r_sb[:], oacc[CQ:CQ + 1, ts(j, 512)])
                        rb = ps_epi.tile([CQ, 512], f32, tag="rb")
                        nc.tensor.matmul(rb[:], ones_sb[:], r_sb[:],
                                         start=True, stop=True)
                        o_sb = epi_sp.tile([CQ, 512], f32, tag="o")
                        nc.scalar.copy(o_sb[:], oacc[0:CQ, ts(j, 512)])
                        on_sb = epi_sp.tile([CQ, 512], f32r, tag="on")
                        nc.vector.tensor_mul(on_sb[:], o_sb[:], rb[:])
                        fp = ps_epi.tile([CIN, 512], f32, tag="f")
                        nc.tensor.matmul(fp[:], woutT_r[:], on_sb[:],
                                         start=True, stop=True)
                        ot = out_pool.tile([CIN, 512], f32)
                        nc.vector.tensor_add(ot[:], fp[:], res_sb[:, ts(j, 512)])
                        nc.sync.dma_start(out_d.ap()[:, ts(j, 512)], ot[:])

    nc.compile()
    return nc


def get_nc():
    if "nc" not in _CACHE:
        _CACHE["nc"] = _build()
    return _CACHE["nc"]


def make_in_maps(input, quary, gn_gamma, gn_beta, wq, wkv, wout, bout):
    input = np.ascontiguousarray(np.asarray(input, dtype=np.float32))
    quary = np.ascontiguousarray(np.asarray(quary, dtype=np.float32))
    wkvT = np.ascontiguousarray(np.asarray(wkv, dtype=np.float32).T)
    wqT = np.ascontiguousarray(np.asarray(wq, dtype=np.float32).T)
    woutT = np.ascontiguousarray(np.asarray(wout, dtype=np.float32).T)
    gam = np.ascontiguousarray(np.asarray(gn_gamma, np.float32).reshape(CIN, 1))
    bet = np.ascontiguousarray(np.asarray(gn_beta, np.float32).reshape(CIN, 1))
    bo = np.ascontiguousarray(np.asarray(bout, np.float32).reshape(CIN, 1))
    gmap = np.zeros((CIN, GRP), np.float32)
    gmap[np.arange(CIN), np.arange(CIN) // (CIN // GRP)] = 1.0
    gmapT = np.ascontiguousarray(gmap.T)

    in_maps = []
    for core in range(N_CORES):
        b, half = divmod(core, 2)
        xb = input[b].reshape(CIN, 2, SQ)
        x_perm = np.ascontiguousarray(
            np.concatenate([xb[:, half], xb[:, 1 - half]], axis=1))
        qh = np.ascontiguousarray(quary[b].reshape(CQ, 2, SQ)[:, half])
        in_maps.append(dict(x=x_perm, qry=qh, wkvT=wkvT, wqT=wqT, woutT=woutT,
                            gam=gam, bet=bet, bout=bo, gmap=gmap,
                            gmapT=gmapT))
    return in_maps


def gather_out(results):
    out = np.empty((B, CIN, HW), np.float32)
    for core in range(N_CORES):
        b, half = divmod(core, 2)
        out[b, :, half * SQ:(half + 1) * SQ] = results[core]["out"]
    return out.reshape(B, CIN, H, W)


def kernel(input, quary, gn_gamma, gn_beta, wq, wkv, wout, bout,
           _trace=False):
    global LAST_EXEC_NS
    nc = get_nc()
    in_maps = make_in_maps(input, quary, gn_gamma, gn_beta, wq, wkv, wout, bout)
    res = run_bass_kernel_spmd(nc, in_maps, list(range(N_CORES)), trace=_trace)
    LAST_EXEC_NS = res.exec_time_ns
    return gather_out(res.results)
